# revision 2
# baseline (speedup 1.0000x reference)
"""v5: 4-way 64x64 PE tiling, fp16 both convs, K=64 taps unpacked.

PE in 64x64 mode = 4 independent tiles T0/T2/T8/T10: (row_grp, col_grp) in
{(0,0),(0,64),(64,0),(64,64)}; four spatial tiles stream concurrently, full
128x128 array utilization at K=64, M=64. All matmuls uniform (64,64) tile
size (no mode switches). Moving planes + weights duplicated into both SBUF
partition halves (rows quadrant must match the moving/stationary source).

Per timestep: conv1 27 taps x 8 spatial tiles = 54 pass-slots, conv2 same.
Quantized planes: single duplicated padded tile per plane (qdup), written by
vector-engine copies (gpsimd tensor_copy measured 1.5us -- never use it).

Epilogue algebra (fewer DVE ops):
  conv1: r1=Relu(s1*ps+b1*s1) [act]; m1=min(r1+M, M+127) [DVE];
         q16=m1-M [act, fp16 out]
  conv2: a2=Relu(s2*ps + b2*s2+127) [act]; uy=min(a2+(M-127), M+127) [DVE]
         ax=Relu(s2*x + 127) [act];        yx=min(ax-M, -M+254) [DVE]
         z=uy+yx [gpsimd]; out=Relu(inv_s2*z - 127*inv_s2) [act]
  (uy = M + clip(round(res*s2)); yx = -M+127 + clip(round(x*s2)))
"""

import numpy as np
import concourse.mybir as mybir
from concourse import bacc
from concourse.tile import TileContext
from concourse.bass_utils import run_bass_kernel_spmd

F16 = mybir.dt.float16
F32 = mybir.dt.float32

MANTISA_BIT = 8.0
MAGIC = 12582912.0

N, C, T, H, W = 8, 64, 16, 56, 56
TP, HP, WP = T + 2, H + 2, W + 2
PLANE = HP * WP
SLICE = H * W
ROWS = 7
NT = ROWS * W  # 392
NTILES = H // ROWS  # 8
NG = NTILES // 4  # 2 groups of 4 spatial tiles

_COMPILED = None

TPOS = [(0, 0), (0, 64), (64, 0), (64, 64)]


def _border_memset(nc, tile):
    v = tile[:].rearrange("p (h w) -> p h w", w=WP)
    nc.gpsimd.memset(v[:, 0, :], 0.0)
    nc.gpsimd.memset(v[:, HP - 1, :], 0.0)
    nc.gpsimd.memset(v[:, 1 : HP - 1, 0], 0.0)
    nc.gpsimd.memset(v[:, 1 : HP - 1, WP - 1], 0.0)


def _build():
    nc = bacc.Bacc()
    xpad_d = nc.declare_dram_parameter("xpad", [C, TP, PLANE], F16, isOutput=False)
    x32_d = nc.declare_dram_parameter("x32", [128, T, SLICE // 2], F32, isOutput=False)
    w1_d = nc.declare_dram_parameter("w1p", [128, 27 * 64], F16, isOutput=False)
    w2_d = nc.declare_dram_parameter("w2p", [128, 27 * 64], F16, isOutput=False)
    coeff_d = nc.declare_dram_parameter("coeff", [128, 8], F32, isOutput=False)
    out_d = nc.declare_dram_parameter("out", [C, T * SLICE], F32, isOutput=True)

    def pview(ap):
        return ap.rearrange("p (h w) -> p h w", w=WP)

    with TileContext(nc) as tc:
        with (
            tc.tile_pool(name="big", bufs=1) as bigpool,
            tc.tile_pool(name="xd", bufs=5) as xpool,
            tc.tile_pool(name="qd", bufs=4) as qpool,
            tc.tile_pool(name="x3", bufs=2) as x3pool,
            tc.tile_pool(name="small", bufs=4) as spool,
            tc.tile_pool(name="ps1", bufs=4, space="PSUM") as ps1pool,
            tc.tile_pool(name="ps2", bufs=4, space="PSUM") as ps2pool,
        ):
            w1 = bigpool.tile([128, 27 * 64], F16, tag="w1")
            nc.sync.dma_start(out=w1[:], in_=w1_d[:])
            w2 = bigpool.tile([128, 27 * 64], F16, tag="w2")
            nc.sync.dma_start(out=w2[:], in_=w2_d[:])
            coeff = bigpool.tile([128, 8], F32, tag="coeff")
            nc.sync.dma_start(out=coeff[:], in_=coeff_d[:])

            s1 = coeff[:, 0:1]
            b1s1 = coeff[:, 1:2]
            s2 = coeff[:, 2:3]
            b2s2p = coeff[:, 3:4]  # b2*s2 + 127
            inv_s2 = coeff[:, 4:5]
            bout = coeff[:, 5:6]  # -127*inv_s2
            negM = coeff[:, 6:7]  # -MAGIC
            c127 = coeff[:, 7:8]  # 127.0

            xdup = {}

            def load_x(s):
                # xdup[s]: both halves = x_pad plane s
                xt_ = xpool.tile([128, PLANE], F16, tag="xdup")
                nc.sync.dma_start(out=xt_[0:64, :], in_=xpad_d[:, s, :])
                nc.sync.dma_start(out=xt_[64:128, :], in_=xpad_d[:, s, :])
                xdup[s] = xt_

            for s in range(3):
                load_x(s)

            # qdup[k]: quantized plane k-1 in both halves (padded); qdup[0]=0
            qdup = {}
            qd0_ = qpool.tile([128, PLANE], F16, tag="qdup")
            nc.gpsimd.memset(qd0_[:], 0.0)
            qdup[0] = qd0_

            def mm_group(wtile, ps_pair, planes, g, n_taps):
                # 4 spatial tiles j=4g..4g+3 on PE tiles T0,T2,T8,T10
                for i in range(n_taps):
                    kd, kh, kw = i // 9, (i // 3) % 3, i % 3
                    pv = planes[kd]
                    wsl_lo = wtile[0:64, 64 * i : 64 * i + 64]
                    wsl_hi = wtile[64:128, 64 * i : 64 * i + 64]
                    for q in range(4):
                        r0 = (4 * g + q) * ROWS
                        half = q // 2  # 0: SBUF partitions 0-63, 1: 64-127
                        ps = ps_pair[half]
                        out_ap = ps[0:64, :] if q % 2 == 0 else ps[64:128, :]
                        mv = pv[64 * half : 64 * half + 64,
                                r0 + kh : r0 + kh + ROWS, kw : kw + W]
                        nc.tensor.matmul(
                            out_ap,
                            wsl_hi if half else wsl_lo,
                            mv,
                            start=(i == 0), stop=(i == n_taps - 1),
                            tile_position=TPOS[q],
                            skip_group_check=True,
                        )

            for t in range(T + 1):
                if t < T:
                    if t + 3 <= TP - 1:
                        load_x(t + 3)
                    qd_ = qpool.tile([128, PLANE], F16, tag="qdup")
                    _border_memset(nc, qd_)
                    qdup[t + 1] = qd_
                    qn_v = pview(qdup[t + 1][:])
                    planes1 = [pview(xdup[t + kd][:]) for kd in range(3)]
                    for g in range(NG):
                        psA = ps1pool.tile([128, NT], F32, tag="ps1")
                        psB = ps1pool.tile([128, NT], F32, tag="ps1")
                        mm_group(w1, (psA, psB), planes1, g, 27)
                        for b, ps in enumerate((psA, psB)):
                            r1 = spool.tile([128, NT], F32, tag="r1")
                            nc.scalar.activation(
                                r1[:], ps[:], mybir.ActivationFunctionType.Relu,
                                bias=b1s1, scale=s1,
                            )
                            m1 = spool.tile([128, NT], F32, tag="m1")
                            nc.vector.tensor_scalar(
                                out=m1[:], in0=r1[:],
                                scalar1=MAGIC, scalar2=MAGIC + 127.0,
                                op0=mybir.AluOpType.add, op1=mybir.AluOpType.min,
                            )
                            q16 = spool.tile([128, NT], F16, tag="q16")
                            nc.scalar.activation(
                                q16[:], m1[:], mybir.ActivationFunctionType.Identity,
                                bias=negM, scale=1.0,
                            )
                            # placements: plane t interior rows of tiles
                            # j = 4g+2b (q16 low) and 4g+2b+1 (q16 high)
                            for h in range(2):
                                qv = q16[64 * h : 64 * h + 64, :].rearrange(
                                    "p (r w) -> p r w", w=W
                                )
                                rr = 1 + (4 * g + 2 * b + h) * ROWS
                                nc.vector.tensor_copy(
                                    qn_v[0:64, rr : rr + ROWS, 1 : 1 + W], qv
                                )
                                nc.vector.tensor_copy(
                                    qn_v[64:128, rr : rr + ROWS, 1 : 1 + W], qv
                                )

                if t >= 1:
                    u = t - 1
                    x32 = x3pool.tile([128, SLICE // 2], F32, tag="x32")
                    nc.sync.dma_start(out=x32[:], in_=x32_d[:, u, :])
                    n_taps = 27 if u + 2 <= T else 18
                    planes2 = [pview(qdup[u + kd][:]) for kd in range(3 if n_taps == 27 else 2)]
                    if n_taps == 18:
                        planes2.append(None)
                    for g in range(NG):
                        psA = ps2pool.tile([128, NT], F32, tag="ps2")
                        psB = ps2pool.tile([128, NT], F32, tag="ps2")
                        mm_group(w2, (psA, psB), planes2, g, n_taps)
                        for b, ps in enumerate((psA, psB)):
                            p2 = 2 * g + b  # pair index: tiles 4g+2b, 4g+2b+1
                            a2 = spool.tile([128, NT], F32, tag="a2")
                            nc.scalar.activation(
                                a2[:], ps[:], mybir.ActivationFunctionType.Relu,
                                bias=b2s2p, scale=s2,
                            )
                            uy = spool.tile([128, NT], F32, tag="uy")
                            nc.vector.tensor_scalar(
                                out=uy[:], in0=a2[:],
                                scalar1=MAGIC - 127.0, scalar2=MAGIC + 127.0,
                                op0=mybir.AluOpType.add, op1=mybir.AluOpType.min,
                            )
                            ax = spool.tile([128, NT], F32, tag="ax")
                            nc.scalar.activation(
                                ax[:], x32[:, p2 * NT : (p2 + 1) * NT],
                                mybir.ActivationFunctionType.Relu,
                                bias=c127, scale=s2,
                            )
                            yx = spool.tile([128, NT], F32, tag="yx")
                            nc.vector.tensor_scalar(
                                out=yx[:], in0=ax[:],
                                scalar1=-MAGIC, scalar2=-MAGIC + 254.0,
                                op0=mybir.AluOpType.add, op1=mybir.AluOpType.min,
                            )
                            z = spool.tile([128, NT], F32, tag="z")
                            nc.gpsimd.tensor_add(z[:], uy[:], yx[:])
                            o_sb = spool.tile([128, NT], F32, tag="osb")
                            nc.scalar.activation(
                                o_sb[:], z[:], mybir.ActivationFunctionType.Relu,
                                bias=bout, scale=inv_s2,
                            )
                            offA = u * SLICE + (4 * g + 2 * b) * NT
                            offB = u * SLICE + (4 * g + 2 * b + 1) * NT
                            nc.sync.dma_start(
                                out=out_d[:, offA : offA + NT], in_=o_sb[0:64, :]
                            )
                            nc.sync.dma_start(
                                out=out_d[:, offB : offB + NT], in_=o_sb[64:128, :]
                            )
    nc.compile()
    return nc


def _host_pack(x, w1, b1, w2, b2, exp1, exp2):
    scale1 = np.exp2(MANTISA_BIT - 1.0 - exp1).astype(np.float32)
    scale2 = np.exp2(MANTISA_BIT - 1.0 - exp2).astype(np.float32)

    def pack_w(wt):
        # wt: [kd,kh,kw,i,o] fp32 -> [128, 27*64] fp16, dup along partitions
        p = wt.reshape(27, 64, 64)
        p = np.ascontiguousarray(np.transpose(p, (1, 0, 2))).reshape(64, 27 * 64)
        return np.concatenate([p, p], axis=0).astype(np.float16)

    w1t = np.transpose(w1, (2, 3, 4, 1, 0)).astype(np.float32)
    w1p = pack_w(w1t)
    w2f = (w2 / scale1[None, :, None, None, None]).astype(np.float32)
    w2t = np.transpose(w2f, (2, 3, 4, 1, 0)).astype(np.float32)
    w2p = pack_w(w2t)

    c64 = np.zeros((64, 8), dtype=np.float32)
    c64[:, 0] = scale1
    c64[:, 1] = b1 * scale1
    c64[:, 2] = scale2
    c64[:, 3] = b2 * scale2 + 127.0
    c64[:, 4] = 1.0 / scale2
    c64[:, 5] = -127.0 / scale2
    c64[:, 6] = -MAGIC
    c64[:, 7] = 127.0
    coeff = np.concatenate([c64, c64], axis=0)

    shared = {"w1p": w1p, "w2p": w2p, "coeff": coeff}
    in_maps = []
    for n in range(N):
        xp = np.pad(x[n], ((0, 0), (1, 1), (1, 1), (1, 1))).astype(np.float16)
        m = dict(shared)
        m["xpad"] = np.ascontiguousarray(xp.reshape(C, TP, PLANE))
        xt = x[n].reshape(C, T, NTILES, NT)
        x32 = np.concatenate([xt[:, :, 0::2, :], xt[:, :, 1::2, :]], axis=0)
        m["x32"] = np.ascontiguousarray(
            x32.reshape(128, T, SLICE // 2).astype(np.float32)
        )
        in_maps.append(m)
    return in_maps


def kernel(x, w1, b1, w2, b2, exp1, exp2):
    global _COMPILED
    x = np.asarray(x, dtype=np.float32)
    w1 = np.asarray(w1, dtype=np.float32)
    b1 = np.asarray(b1, dtype=np.float32)
    w2 = np.asarray(w2, dtype=np.float32)
    b2 = np.asarray(b2, dtype=np.float32)
    exp1 = np.asarray(exp1, dtype=np.float32)
    exp2 = np.asarray(exp2, dtype=np.float32)
    if _COMPILED is None:
        _COMPILED = _build()
    in_maps = _host_pack(x, w1, b1, w2, b2, exp1, exp2)
    res = run_bass_kernel_spmd(_COMPILED, in_maps, core_ids=list(range(N)))
    out = np.stack([res.results[i]["out"].reshape(C, T, H, W) for i in range(N)])
    return out.astype(np.float32)
